# revision 14
# baseline (speedup 1.0000x reference)
"""MinGRU cell kernel v2 for Trainium2, 8 NeuronCores, data-parallel over B.

Math (per core = one batch element):
    z = Wz.x          [H, T] (PSUM)      a = sigmoid(-z - bz) = 1 - gate
    u' = Wh.x         [H, T] (PSUM, biasless: bh folded out on host)
    m_t = (d_t + m_{t-1}) * a_t    with d_t = u'_{t-1} - u'_t, m_{-1} = h0 - bh
    s_t = m_t + u'_t               (stored bf16, h = s + bh added on host)

This substitution (m = h-hat - u') turns the gate-combine into:
  - ONE sigmoid per (chunk, hh) on ACT (the only mandatory PSUM z read)
  - ONE u' eviction PSUM->SBUF bf16 per chunk (ACT copy or DVE copy)
  - d  = shifted-difference: plain tensor_tensor sub (DVE 2x rate / Pool)
  - scan (DVE, 1x - ISA rate)
  - s  = m + u' add (DVE 2x / Pool)
All heavy layout work is host-side: x pre-transposed+bf16, weights packed
in lhsT layout bf16, output [hh, p, t] bf16 upcast+bias+transposed on host.
"""

import sys

sys.path.insert(0, "/opt/trn_rl_repo")

from contextlib import ExitStack

import numpy as np
import ml_dtypes

import bass_rust
import concourse.bass as bass
import concourse.mybir as mybir
import concourse.tile as tile
from concourse.bass_utils import run_bass_kernel_spmd

B, T, D, H = 8, 4096, 256, 256
P = 128
TC = 512          # t-chunk (PSUM bank = 512 fp32)
NCH = T // TC     # 8 chunks
F32 = mybir.dt.float32
BF16 = mybir.dt.bfloat16
AOP = mybir.AluOpType
SIG = mybir.ActivationFunctionType.Sigmoid
CPY = mybir.ActivationFunctionType.Copy

N_CORES = 8

# Per-chunk engine assignment for the three movable elementwise passes.
# "act" = scalar engine, "dve" = vector engine, "pool" = gpsimd.
EVICT_ENG = ["dve", "dve", "mix", "act", "act", "act", "act", "act"]
D_ENG     = ["dve"] * 8
ADD_ENG   = ["pool"] * 7 + ["mix"]
WU = 128          # warmup length for chain decoupling (unused when no starts)
CHAIN_STARTS = ()
ZP_BUFS = 2
UP_BUFS = 3
PAIR_SIG = False  # sigmoid over 2-chunk PSUM z tiles (forces UP_BUFS=2)
FINAL_STORE = ("sync", "scalar")  # queue per hh for the c=6,7 stores
HEAD_SPLIT = 0    # chunks < this get per-hh evict/d (measured: not helpful)


def _split_sync_waits(nc, max_waits=1):
    """walrus CoreV3 accepts at most 1 sync-wait per instruction; move
    excess waits onto preceding same-engine NoOps."""
    n = 0
    cnt = [0]
    for f in nc.m.functions:
        for bb in f.blocks:
            out = []
            changed = False
            for inst in bb.instructions:
                si = inst.sync_info
                if si is not None and si.on_wait and len(si.on_wait) > max_waits:
                    waits = list(si.on_wait)
                    extra, keep = waits[:-max_waits], waits[-max_waits:]
                    for j in range(0, len(extra), max_waits):
                        cnt[0] += 1
                        nop = bass_rust.InstNoOp(
                            name=f"I-waitsplit-{cnt[0]}", engine=inst.engine
                        )
                        nop.sync_info = mybir.SyncInfo(
                            on_wait=extra[j : j + max_waits], on_update=[]
                        )
                        out.append(nop)
                    inst.sync_info = mybir.SyncInfo(
                        on_wait=keep, on_update=list(si.on_update or [])
                    )
                    changed = True
                    n += 1
                out.append(inst)
            if changed:
                bb.instructions = out
    return n


def build_nc(reps=1):
    nc = bass.Bass()
    xt = nc.dram_tensor("xt", [2, P, T], BF16, kind="ExternalInput")
    wt = nc.dram_tensor("wt", [P, 2, 2, 2, P], BF16, kind="ExternalInput")
    cst = nc.dram_tensor("cst", [P, 4], F32, kind="ExternalInput")
    ot = nc.dram_tensor("ot", [2, P, T], BF16, kind="ExternalOutput")
    tens = (xt, wt, cst, ot)

    with tile.TileContext(nc) as tc, ExitStack() as ctx:
        pools = {
            "consts": ctx.enter_context(tc.tile_pool(name="consts", bufs=1)),
            "xc": ctx.enter_context(tc.tile_pool(name="xc", bufs=3)),
            "a": ctx.enter_context(tc.tile_pool(name="a", bufs=4)),
            "d": ctx.enter_context(tc.tile_pool(name="d", bufs=2)),
            "m": ctx.enter_context(tc.tile_pool(name="m", bufs=2)),
            "wu": ctx.enter_context(tc.tile_pool(name="wu", bufs=4)),
            "zp": ctx.enter_context(
                tc.tile_pool(name="zp", bufs=ZP_BUFS, space="PSUM")
            ),
            "up": ctx.enter_context(
                tc.tile_pool(name="up", bufs=UP_BUFS, space="PSUM")
            ),
        }
        for _rep in range(reps):
            _emit(nc, pools, tens)

    _split_sync_waits(nc)
    return nc


def _emit(nc, pools, tens):
    xt, wt, cst, ot = tens
    consts = pools["consts"]

    # ---- constants ---------------------------------------------------
    # cst cols: 0,1 = m_init per hh (h0 - bh); 2,3 = -bz per hh
    cst_sb = consts.tile([P, 4], F32, tag="cst")
    nc.scalar.dma_start(out=cst_sb, in_=cst[:, :])
    # weights: [p_d, g, hh, kk, n_h] bf16, single DMA (4KB/partition)
    wt_sb = consts.tile([P, 2, 2, 2, P], BF16, tag="wt")
    nc.gpsimd.dma_start(
        out=wt_sb.rearrange("p g h k n -> p (g h k n)"),
        in_=wt[:, :, :, :, :].rearrange("p g h k n -> p (g h k n)"),
    )
    # persistent big buffers
    u_all = consts.tile([P, 2, T + 8], BF16, tag="u_all")  # [p, hh, 1+t]
    h_all = consts.tile([P, 2, T], BF16, tag="h_all")
    # u'_{-1} = 0
    nc.vector.memset(u_all[:, :, 0:1], 0.0)
    # preload the sigmoid act table off the critical path (1283ns)
    warm = consts.tile([P, 1], BF16, tag="warm")
    nc.scalar.activation(warm, cst_sb[:, 0:1], SIG)

    # ---- x chunk loads ----------------------------------------------
    xtiles = [None] * NCH

    def load_x(c, split=False):
        t = pools["xc"].tile([P, 2, TC], BF16, tag="xn")
        if split:
            # kk halves on separate queues: DGE latencies overlap and
            # the kk=0 matmul can start after half the bytes land
            for kk, eng in ((0, nc.sync), (1, nc.scalar)):
                eng.dma_start(
                    out=t[:, kk, :],
                    in_=xt[kk, :, c * TC : (c + 1) * TC],
                )
        else:
            nc.sync.dma_start(
                out=t,
                in_=xt[:, :, c * TC : (c + 1) * TC].rearrange(
                    "k p t -> p k t"
                ),
            )
        xtiles[c] = t

    load_x(0, split=True)
    load_x(1, split=True)

    m_prev = None
    wu_tiles = {}
    pending = []
    for c in range(NCH):
        if c + 2 < NCH:
            load_x(c + 2)
        xtile = xtiles[c]
        # ---- PE: z and u' GEMMs -------------------------------------
        u_ps = pools["up"].tile([P, 2, TC], F32, tag="u_ps")
        for hh in range(2):
            for kk in range(2):
                nc.tensor.matmul(
                    u_ps[:, hh, :], wt_sb[:, 1, hh, kk, :], xtile[:, kk, :],
                    start=(kk == 0), stop=(kk == 1),
                )
        if PAIR_SIG:
            if c % 2 == 0:
                zpair0 = pools["zp"].tile([P, 2, TC], F32, tag="z_pair")
                zpair1 = pools["zp"].tile([P, 2, TC], F32, tag="z_pair")
                apair = pools["a"].tile([P, 2, 2 * TC], BF16, tag="a_pair")
                pair_state = ([zpair0, zpair1], apair)
            zpair, apair = pair_state
            for hh in range(2):
                for kk in range(2):
                    nc.tensor.matmul(
                        zpair[hh][:, c % 2, :], wt_sb[:, 0, hh, kk, :],
                        xtile[:, kk, :], start=(kk == 0), stop=(kk == 1),
                    )
            a_sb = [apair[:, hh, (c % 2) * TC : (c % 2 + 1) * TC] for hh in range(2)]
            if c % 2 == 1:
                for hh in range(2):
                    nc.scalar.activation(
                        apair[:, hh, :],
                        zpair[hh].rearrange("p i t -> p (i t)"), SIG,
                        bias=cst_sb[:, 2 + hh : 3 + hh], scale=-1.0,
                    )
        else:
            z_ps = []
            for hh in range(2):
                z = pools["zp"].tile([P, TC], F32, tag="z_ps")
                for kk in range(2):
                    nc.tensor.matmul(
                        z, wt_sb[:, 0, hh, kk, :], xtile[:, kk, :],
                        start=(kk == 0), stop=(kk == 1),
                    )
                z_ps.append(z)
            a_sb = []
            for hh in range(2):
                a_t = pools["a"].tile([P, TC], BF16, tag="a_sb")
                nc.scalar.activation(
                    a_t, z_ps[hh], SIG,
                    bias=cst_sb[:, 2 + hh : 3 + hh], scale=-1.0,
                )
                a_sb.append(a_t)
        # ---- evict u' PSUM -> u_all -------------------------------
        uslot = u_all[:, :, 1 + c * TC : 1 + (c + 1) * TC]
        ev = EVICT_ENG[c]
        if ev == "act":
            nc.scalar.activation(uslot, u_ps, CPY)
        elif ev == "dve":
            nc.vector.tensor_copy(uslot, u_ps)
        else:  # mix: hh0 on ACT, hh1 on DVE
            nc.scalar.activation(uslot[:, 0, :], u_ps[:, 0, :], CPY)
            nc.vector.tensor_copy(uslot[:, 1, :], u_ps[:, 1, :])
        # ---- d = u'_{t-1} - u'_t (both hh fused) --------------------
        d_t = pools["d"].tile([P, 2, TC], BF16, tag="d_sb")
        d_eng = {"dve": nc.vector, "pool": nc.gpsimd}[D_ENG[c]]
        if c < HEAD_SPLIT and D_ENG[c] == "dve":
            for hh in range(2):
                d_eng.tensor_tensor(
                    d_t[:, hh, :],
                    u_all[:, hh, c * TC : (c + 1) * TC],
                    u_all[:, hh, 1 + c * TC : 1 + (c + 1) * TC],
                    AOP.subtract,
                )
        else:
            d_eng.tensor_tensor(
                d_t,
                u_all[:, :, c * TC : (c + 1) * TC],
                u_all[:, :, 1 + c * TC : 1 + (c + 1) * TC],
                AOP.subtract,
            )
        def tail_work(c, d_t, a_sb):
            nonlocal m_prev
            # ---- scan: m = (d + m_prev) * a -------------------------
            m_t = pools["m"].tile([P, 2, TC], BF16, tag="m_sb")
            for hh in range(2):
                if c == 0:
                    init = cst_sb[:, hh : hh + 1]
                elif c in CHAIN_STARTS:
                    init = wu_tiles[(c, hh)][:, WU - 1 : WU]
                else:
                    init = m_prev[:, hh, TC - 1 : TC]
                nc.vector.tensor_tensor_scan(
                    m_t[:, hh, :], d_t[:, hh, :], a_sb[hh], init,
                    AOP.add, AOP.mult,
                )
                if (c + 1) in CHAIN_STARTS:
                    wu = pools["wu"].tile([P, WU], BF16, tag="wu")
                    nc.vector.tensor_tensor_scan(
                        wu, d_t[:, hh, TC - WU :], a_sb[hh][:, TC - WU :],
                        0.0, AOP.add, AOP.mult,
                    )
                    wu_tiles[(c + 1, hh)] = wu
            m_prev = m_t
            # ---- s = m + u' -> h_all --------------------------------
            if ADD_ENG[c] == "mix":
                for hh, eng in ((0, nc.gpsimd), (1, nc.vector)):
                    eng.tensor_tensor(
                        h_all[:, hh, c * TC : (c + 1) * TC],
                        m_t[:, hh, :],
                        u_all[:, hh, 1 + c * TC : 1 + (c + 1) * TC],
                        AOP.add,
                    )
            else:
                add_eng = {"dve": nc.vector, "pool": nc.gpsimd}[ADD_ENG[c]]
                add_eng.tensor_tensor(
                    h_all[:, :, c * TC : (c + 1) * TC],
                    m_t,
                    u_all[:, :, 1 + c * TC : 1 + (c + 1) * TC],
                    AOP.add,
                )
            # ---- stores ---------------------------------------------
            if c in (1, 3, 5):
                lo, hi = (c - 1) * TC, (c + 1) * TC
                for hh in range(2):
                    nc.sync.dma_start(
                        out=ot[hh, :, lo:hi],
                        in_=h_all[:, hh, lo:hi],
                    )
            elif c in (6, 7):
                lo, hi = c * TC, (c + 1) * TC
                for hh in range(2):
                    eng = getattr(nc, FINAL_STORE[hh])
                    eng.dma_start(
                        out=ot[hh, :, lo:hi],
                        in_=h_all[:, hh, lo:hi],
                    )

        if PAIR_SIG:
            pending.append((c, d_t, a_sb))
            if c % 2 == 1:
                for args in pending:
                    tail_work(*args)
                pending = []
        else:
            tail_work(c, d_t, a_sb)


_NC_CACHE = {}


def _get_nc(reps=1):
    if reps not in _NC_CACHE:
        _NC_CACHE[reps] = build_nc(reps)
    return _NC_CACHE[reps]


def _prep_inputs(x, h0, Wz, bz, Wh, bh):
    """Host-side packing; returns per-core in_maps."""
    x = np.asarray(x, dtype=np.float32)
    h0 = np.asarray(h0, dtype=np.float32)
    Wz = np.asarray(Wz, dtype=np.float32)
    bz = np.asarray(bz, dtype=np.float32)
    Wh = np.asarray(Wh, dtype=np.float32)
    bh = np.asarray(bh, dtype=np.float32)

    # weights: wt[p_d, g, hh, kk, n_h] = W_g[hh*128+n_h, kk*128+p_d]
    wt = np.empty((P, 2, 2, 2, P), dtype=ml_dtypes.bfloat16)
    Wz4 = Wz.reshape(2, P, 2, P)  # [hh, n_h, kk, p_d]
    Wh4 = Wh.reshape(2, P, 2, P)
    for g, W4 in enumerate((Wz4, Wh4)):
        # -> [p_d, hh, kk, n_h]
        wt[:, g] = W4.transpose(3, 0, 2, 1).astype(ml_dtypes.bfloat16)

    # consts: cols 0,1 = h0 - bh per hh; 2,3 = -bz per hh
    m_init = (h0 - bh[None, :]).reshape(B, 2, P)  # [b, hh, p]
    nbz = (-bz).reshape(2, P)  # [hh, p]

    in_maps = []
    for b in range(N_CORES):
        xT = np.ascontiguousarray(x[b].T).astype(ml_dtypes.bfloat16)  # [D, T]
        cstb = np.empty((P, 4), np.float32)
        cstb[:, 0] = m_init[b, 0]
        cstb[:, 1] = m_init[b, 1]
        cstb[:, 2] = nbz[0]
        cstb[:, 3] = nbz[1]
        in_maps.append(
            {
                "xt": np.ascontiguousarray(xT.reshape(2, P, T)),
                "wt": wt,
                "cst": cstb,
            }
        )
    return in_maps


def _post_output(res, bh):
    """res: list of {'ot': [2, P, T] bf16}; returns [B, T, H] f32."""
    bh = np.asarray(bh, dtype=np.float32).reshape(2, P)
    out = np.empty((len(res), T, H), np.float32)
    for b in range(len(res)):
        o = res[b]["ot"].astype(np.float32) + bh[:, :, None]  # [hh, p, t]
        out[b] = o.transpose(2, 0, 1).reshape(T, H)
    return out


def kernel(x, h0, Wz, bz, Wh, bh):
    nc = _get_nc(1)
    in_maps = _prep_inputs(x, h0, Wz, bz, Wh, bh)
    res = run_bass_kernel_spmd(nc, in_maps, list(range(N_CORES))).results
    return _post_output(res, bh)
